# revision 15
# baseline (speedup 1.0000x reference)
"""MultiHeadAttn1D (N=4, C=256, T=2048, H=8, dk=dv=32) Trainium2 Bass kernel.

Sharding: 8 cores = 4 batches x 2 head-groups (4 heads each).
Per core: x_n [256,2048] -> q,k [128,2048] ([4h*32d, t]); augmented value
tiles vT_aug per (s-tile i, head h): [128s, 64] = [vT | ones] so one matmul
accumulates both PV and the softmax denominator (rows 32-63 = sums,
replicated); scores^T per (tq-chunk, s-tile) via K=32 row-tiled fp32r
matmuls -> PSUM; exp on ScalarE PSUM->SBUF (bottleneck engine, kept
saturated); normalize with DVE reciprocal+mul; DMA out.
"""

import sys

if "/opt/trn_rl_repo" not in sys.path:
    sys.path.insert(0, "/opt/trn_rl_repo")

import numpy as np

N_CORES = 8
C = 256          # in channels
T = 2048         # sequence length
HG = 4           # heads per core
DK = 32          # head dim
OC = 128         # output channels per core (HG * DK)
TQ = 512         # tq chunk width (1 psum bank)
NCHUNK = T // TQ          # 4
NST = T // 128            # 16 s-tiles
INV_SQRT_DK = 1.0 / np.sqrt(DK)
ONE_U32 = 0x3F800000      # 1.0f bit pattern
ONES_COL = (32, 64, 32, 64)  # ones col range (sums rows) per head in vT_aug block

USE_FP32R = True
TRACE = False
LAST = {}

_CACHE = {}


def _build_module():
    import contextlib

    from concourse import bacc, mybir
    import concourse.tile as tile

    f32 = mybir.dt.float32
    u32 = mybir.dt.uint32
    f32r = mybir.dt.float32r if USE_FP32R else mybir.dt.float32
    Exp = mybir.ActivationFunctionType.Exp

    nc = bacc.Bacc(
        "TRN2",
        target_bir_lowering=False,
        debug=False,
        num_devices=N_CORES,
    )

    x_d = nc.dram_tensor("x", [C, T], f32, kind="ExternalInput").ap()
    wqT_d = nc.dram_tensor("wqT", [C, OC], f32, kind="ExternalInput").ap()
    wkT_d = nc.dram_tensor("wkT", [C, OC], f32, kind="ExternalInput").ap()
    wvT_d = nc.dram_tensor("wvT", [C, OC], f32, kind="ExternalInput").ap()
    out_d = nc.dram_tensor("out", [OC, T], f32, kind="ExternalOutput").ap()

    with tile.TileContext(nc) as tc:
        ctx = contextlib.ExitStack()
        with ctx:
            persist = ctx.enter_context(tc.tile_pool(name="persist", bufs=1))
            pt_pool = ctx.enter_context(tc.tile_pool(name="pt", bufs=8))
            epi_pool = ctx.enter_context(tc.tile_pool(name="epi", bufs=2))
            # PSUM: scores 2x[128,1024] (4 banks, also hosts prologue
            # projection tiles via shared tag) + 4 PV banks = 8 banks.
            ps_sc = ctx.enter_context(tc.tile_pool(name="ps_sc", bufs=2, space="PSUM"))
            ps_pv = ctx.enter_context(tc.tile_pool(name="ps_pv", bufs=4, space="PSUM"))

            # ---------------- persistent SBUF ----------------
            wqT_sb = persist.tile([128, C], f32r)      # c-tile j at cols [128j:128j+128] = o
            wkT_sb = persist.tile([128, C], f32r)
            wvT_sb = persist.tile([128, C], f32r)
            x_sb = persist.tile([128, 2 * T], f32r)    # c-tile j at cols [T*j : T*j+T]
            q_sb = persist.tile([128, T], f32r)        # [o=4h*32, t]
            k_sb = persist.tile([128, T], f32r)
            # vT_aug: block b=(4i+h) at cols [128b:128b+128], zero-padded:
            # vT head h s-tile i at cols [32h:32h+32] (-> PV rows 32h..32h+32),
            # ones at cols ONES_COL[h] (-> replicated sums rows), zeros else.
            vT_aug = persist.tile([128, 128 * HG * NST], f32r)
            x_stage = persist.tile([128, 2 * T], f32)
            w_stage = persist.tile([128, 3 * C], f32)

            # zero + ones init of vT_aug on GpSimd (DVE is busy staging x).
            vT_aug_u32 = vT_aug.bitcast(u32)
            nc.gpsimd.memset(vT_aug_u32[:], 0)
            v5 = vT_aug_u32.rearrange("p (i h w) -> p i h w", h=HG, w=128)
            for h in range(HG):
                oc = ONES_COL[h]
                nc.gpsimd.memset(v5[:, :, h, oc:oc + DK], ONE_U32)

            # ---------------- input DMAs + fp32r rounding casts ----------------
            def stage_w(idx, w_sb, w_d):
                for j in range(2):
                    sl = slice(C * idx + 128 * j, C * idx + 128 * (j + 1))
                    nc.sync.dma_start(w_stage[:, sl], w_d[128 * j:128 * (j + 1), :])
                    nc.vector.tensor_copy(w_sb[:, 128 * j:128 * (j + 1)], w_stage[:, sl])

            def stage_x(j, t):
                sl = slice(T * j + TQ * t, T * j + TQ * (t + 1))
                nc.sync.dma_start(x_stage[:, sl], x_d[128 * j:128 * (j + 1), TQ * t:TQ * (t + 1)])
                nc.vector.tensor_copy(x_sb[:, sl], x_stage[:, sl])

            # critical path first: q/k weights + x t-chunk 0
            stage_w(0, wqT_sb, wqT_d)
            stage_w(1, wkT_sb, wkT_d)
            stage_x(0, 0)
            stage_x(1, 0)
            stage_w(2, wvT_sb, wvT_d)
            for t in range(1, NCHUNK):
                stage_x(0, t)
                stage_x(1, t)

            # ---------------- projection helpers ----------------
            def proj_chunk(dst_sb, w_sb, t):
                """q/k projection for t-chunk: out[o, TQ] accumulated over 2 c-tiles."""
                ps = ps_sc.tile([128, TQ], f32, tag="sc", name=f"projps_{dst_sb.name}_{t}")
                for j in range(2):
                    nc.tensor.matmul(
                        ps[:],
                        w_sb[:, 128 * j:128 * (j + 1)],
                        x_sb[:, T * j + TQ * t: T * j + TQ * (t + 1)],
                        start=(j == 0), stop=(j == 1))
                nc.vector.tensor_copy(dst_sb[:, TQ * t:TQ * (t + 1)], ps[:])

            def proj_vT(i):
                """vT for s-tile i: out[t=128, o=128], scatter to vT_aug blocks."""
                ps = ps_sc.tile([128, 128], f32, tag="sc", name=f"vtps_{i}")
                for j in range(2):
                    nc.tensor.matmul(
                        ps[:],
                        x_sb[:, T * j + 128 * i: T * j + 128 * (i + 1)],
                        wvT_sb[:, 128 * j:128 * (j + 1)],
                        start=(j == 0), stop=(j == 1))
                for h in range(HG):
                    b = 4 * i + h
                    nc.vector.tensor_copy(
                        vT_aug[:, 128 * b + 32 * h:128 * b + 32 * h + DK],
                        ps[:, 32 * h:32 * (h + 1)])

            # pre-loop: just enough for the first exps
            proj_chunk(q_sb, wqT_sb, 0)
            proj_chunk(k_sb, wkT_sb, 0)

            # deferred projection work, drip-fed one task per attention iter
            tasks = [
                lambda: [proj_vT(i) for i in range(0, 4)],
                lambda: [proj_vT(i) for i in range(4, 8)],
                lambda: proj_chunk(k_sb, wkT_sb, 1),
                lambda: [proj_vT(i) for i in range(8, 12)],
                lambda: [proj_vT(i) for i in range(12, 16)],
                lambda: proj_chunk(k_sb, wkT_sb, 2),
                lambda: proj_chunk(q_sb, wqT_sb, 1),
                lambda: proj_chunk(k_sb, wkT_sb, 3),
                lambda: proj_chunk(q_sb, wqT_sb, 2),
                lambda: proj_chunk(q_sb, wqT_sb, 3),
            ]
            tasks.reverse()

            # ---------------- attention ----------------
            pv_ps = {}   # (c, h) -> [128, TQ] psum bank; rows 0:32 PV, 32:64 sums
            pending = []  # (c, i, [pT_A, pT_B]) awaiting PV emission

            def emit_pv(c, i, pTs):
                for h in range(HG):
                    pT = pTs[h // 2]
                    sl = pT[:, TQ * (h % 2):TQ * (h % 2 + 1)]
                    b = 4 * i + h
                    nc.tensor.matmul(
                        pv_ps[c, h][:],
                        vT_aug[:, 128 * b:128 * (b + 1)],
                        sl,
                        start=(i == 0), stop=(i == NST - 1))

            def emit_epilogue(c):
                # Evacuate PV rows (aligned) + sums rows (partition-shifted,
                # PSUM-sourced so the shift is legal) -> banks release after
                # these copies; then recip+mul fully in SBUF, all aligned.
                pvc = epi_pool.tile([128, TQ], f32, tag="pvc", name=f"pvc_{c}")
                gat = epi_pool.tile([128, TQ], f32, tag="gat", name=f"gat_{c}")
                for h in range(HG):
                    nc.vector.tensor_copy(pvc[32 * h:32 * (h + 1), :],
                                          pv_ps[c, h][32 * h:32 * (h + 1), :])
                    sr = ONES_COL[h]
                    nc.vector.tensor_copy(gat[32 * h:32 * (h + 1), :],
                                          pv_ps[c, h][sr:sr + DK, :])
                recip = epi_pool.tile([128, TQ], f32, tag="recip", name=f"recip_{c}")
                rscr = epi_pool.tile([128, TQ], f32, tag="rscr", name=f"rscr_{c}")
                nc.vector.reciprocal_approx_accurate(recip[:], gat[:], rscr[:])
                outsb = epi_pool.tile([128, TQ], f32, tag="outsb", name=f"outsb_{c}")
                nc.vector.tensor_mul(outsb[:], pvc[:], recip[:])
                nc.sync.dma_start(out_d[:, TQ * c:TQ * (c + 1)], outsb[:])

            def flush_one():
                c0, i0, pTs = pending.pop(0)
                emit_pv(c0, i0, pTs)
                if i0 == NST - 1:
                    emit_epilogue(c0)

            for c in range(NCHUNK):
                for h in range(HG):
                    pv_ps[c, h] = ps_pv.tile([128, TQ], f32, tag="pv",
                                             name=f"pv_{c}_{h}")
                for i in range(NST):
                    if tasks:
                        tasks.pop()()
                    pTs = []
                    for half in range(2):  # half 0: heads 0,1; half 1: heads 2,3
                        sc = ps_sc.tile([128, 2 * TQ], f32, tag="sc",
                                        name=f"sc_{c}_{i}_{half}")
                        for hh in range(2):
                            h = 2 * half + hh
                            nc.tensor.matmul(
                                sc[:, TQ * hh:TQ * (hh + 1)],
                                k_sb[32 * h:32 * (h + 1), 128 * i:128 * (i + 1)],
                                q_sb[32 * h:32 * (h + 1), TQ * c:TQ * (c + 1)],
                                start=True, stop=True,
                                tile_position=(32 * h, 0))
                        pT = pt_pool.tile([128, 2 * TQ], f32r, tag="pt",
                                          name=f"pt_{c}_{i}_{half}", bufs=8)
                        nc.scalar.activation(pT[:], sc[:], Exp,
                                             scale=float(INV_SQRT_DK))
                        pTs.append(pT)
                    pending.append((c, i, pTs))
                    if len(pending) > 3:
                        flush_one()
            while pending:
                flush_one()

    nc.compile()
    return nc


def _get_module():
    if "nc" not in _CACHE:
        _CACHE["nc"] = _build_module()
    return _CACHE["nc"]


def kernel(x, Wq, Wk, Wv):
    from concourse.bass_utils import run_bass_kernel_spmd

    nc = _get_module()
    x = np.ascontiguousarray(x, dtype=np.float32)
    in_maps = []
    for core in range(N_CORES):
        n, g = divmod(core, 2)
        rows = slice(128 * g, 128 * (g + 1))
        in_maps.append({
            "x": np.ascontiguousarray(x[n]),
            "wqT": np.ascontiguousarray(Wq[rows, :].T),
            "wkT": np.ascontiguousarray(Wk[rows, :].T),
            "wvT": np.ascontiguousarray(Wv[rows, :].T),
        })
    res = run_bass_kernel_spmd(nc, in_maps, core_ids=list(range(N_CORES)), trace=TRACE)
    LAST["res"] = res
    out = np.empty((4, 256, T), dtype=np.float32)
    for core in range(N_CORES):
        n, g = divmod(core, 2)
        out[n, 128 * g:128 * (g + 1), :] = res.results[core]["out"]
    return out


if __name__ == "__main__":
    _build_module()
    print("module built OK")


# revision 16
# speedup vs baseline: 1.0288x; 1.0288x over previous
"""MultiHeadAttn1D (N=4, C=256, T=2048, H=8, dk=dv=32) Trainium2 Bass kernel.

Sharding: 8 cores = 4 batches x 2 head-groups (4 heads each).
Per core: x_n [256,2048] -> q,k [128,2048] ([4h*32d, t]); augmented value
tiles vT_aug per (s-tile i, head h): [128s, 64] = [vT | ones] so one matmul
accumulates both PV and the softmax denominator (rows 32-63 = sums,
replicated); scores^T per (tq-chunk, s-tile) via K=32 row-tiled fp32r
matmuls -> PSUM; exp on ScalarE PSUM->SBUF (bottleneck engine, kept
saturated); normalize with DVE reciprocal+mul; DMA out.
"""

import sys

if "/opt/trn_rl_repo" not in sys.path:
    sys.path.insert(0, "/opt/trn_rl_repo")

import numpy as np

N_CORES = 8
C = 256          # in channels
T = 2048         # sequence length
HG = 4           # heads per core
DK = 32          # head dim
OC = 128         # output channels per core (HG * DK)
TQ = 512         # tq chunk width (1 psum bank)
NCHUNK = T // TQ          # 4
NST = T // 128            # 16 s-tiles
INV_SQRT_DK = 1.0 / np.sqrt(DK)
ONE_U32 = 0x3F800000      # 1.0f bit pattern
# vT_aug block layout per head (pair p=h//2 shares one PSUM bank; zero pad
# columns accumulate zeros into the partner head's rows, which is harmless):
#   h%2==0: vT at cols 0-31  (PV rows 0-31),  ones at cols 32-63  (sums rows 32-63)
#   h%2==1: vT at cols 64-95 (PV rows 64-95), ones at cols 96-127 (sums rows 96-127)
VT_COL = (0, 64, 0, 64)
ONES_COL = (32, 96, 32, 96)

USE_FP32R = True
TRACE = False
LAST = {}

_CACHE = {}


def _build_module():
    import contextlib

    from concourse import bacc, mybir
    import concourse.tile as tile

    f32 = mybir.dt.float32
    u32 = mybir.dt.uint32
    f32r = mybir.dt.float32r if USE_FP32R else mybir.dt.float32
    Exp = mybir.ActivationFunctionType.Exp

    nc = bacc.Bacc(
        "TRN2",
        target_bir_lowering=False,
        debug=False,
        num_devices=N_CORES,
    )

    x_d = nc.dram_tensor("x", [C, T], f32, kind="ExternalInput").ap()
    wqT_d = nc.dram_tensor("wqT", [C, OC], f32, kind="ExternalInput").ap()
    wkT_d = nc.dram_tensor("wkT", [C, OC], f32, kind="ExternalInput").ap()
    wvT_d = nc.dram_tensor("wvT", [C, OC], f32, kind="ExternalInput").ap()
    out_d = nc.dram_tensor("out", [OC, T], f32, kind="ExternalOutput").ap()

    with tile.TileContext(nc) as tc:
        ctx = contextlib.ExitStack()
        with ctx:
            persist = ctx.enter_context(tc.tile_pool(name="persist", bufs=1))
            pt_pool = ctx.enter_context(tc.tile_pool(name="pt", bufs=8))
            epi_pool = ctx.enter_context(tc.tile_pool(name="epi", bufs=2))
            # PSUM: scores 3x[128,1024] (6 banks, also hosts prologue
            # projection tiles via shared tag) + 2 shared PV banks = 8 banks.
            ps_sc = ctx.enter_context(tc.tile_pool(name="ps_sc", bufs=3, space="PSUM"))
            ps_pv = ctx.enter_context(tc.tile_pool(name="ps_pv", bufs=2, space="PSUM"))

            # ---------------- persistent SBUF ----------------
            wqT_sb = persist.tile([128, C], f32r)      # c-tile j at cols [128j:128j+128] = o
            wkT_sb = persist.tile([128, C], f32r)
            wvT_sb = persist.tile([128, C], f32r)
            x_sb = persist.tile([128, 2 * T], f32r)    # c-tile j at cols [T*j : T*j+T]
            q_sb = persist.tile([128, T], f32r)        # [o=4h*32, t]
            k_sb = persist.tile([128, T], f32r)
            # vT_aug: block b=(4i+h) at cols [128b:128b+128], zero-padded per
            # the VT_COL/ONES_COL layout above.
            vT_aug = persist.tile([128, 128 * HG * NST], f32r)
            x_stage = persist.tile([128, 2 * T], f32)
            w_stage = persist.tile([128, 3 * C], f32)

            # zero + ones init of vT_aug on GpSimd (DVE is busy staging x).
            vT_aug_u32 = vT_aug.bitcast(u32)
            nc.gpsimd.memset(vT_aug_u32[:], 0)
            v5 = vT_aug_u32.rearrange("p (i h w) -> p i h w", h=HG, w=128)
            for h in range(HG):
                oc = ONES_COL[h]
                nc.gpsimd.memset(v5[:, :, h, oc:oc + DK], ONE_U32)

            # ---------------- input DMAs + fp32r rounding casts ----------------
            def stage_w(idx, w_sb, w_d):
                for j in range(2):
                    sl = slice(C * idx + 128 * j, C * idx + 128 * (j + 1))
                    nc.sync.dma_start(w_stage[:, sl], w_d[128 * j:128 * (j + 1), :])
                    nc.vector.tensor_copy(w_sb[:, 128 * j:128 * (j + 1)], w_stage[:, sl])

            def stage_x(j, t):
                sl = slice(T * j + TQ * t, T * j + TQ * (t + 1))
                nc.sync.dma_start(x_stage[:, sl], x_d[128 * j:128 * (j + 1), TQ * t:TQ * (t + 1)])
                nc.vector.tensor_copy(x_sb[:, sl], x_stage[:, sl])

            # critical path first: q/k weights + x t-chunk 0
            stage_w(0, wqT_sb, wqT_d)
            stage_w(1, wkT_sb, wkT_d)
            stage_x(0, 0)
            stage_x(1, 0)
            stage_w(2, wvT_sb, wvT_d)
            for t in range(1, NCHUNK):
                stage_x(0, t)
                stage_x(1, t)

            # ---------------- projection helpers ----------------
            def proj_chunk(dst_sb, w_sb, t):
                """q/k projection for t-chunk: out[o, TQ] accumulated over 2 c-tiles."""
                ps = ps_sc.tile([128, TQ], f32, tag="sc", name=f"projps_{dst_sb.name}_{t}")
                for j in range(2):
                    nc.tensor.matmul(
                        ps[:],
                        w_sb[:, 128 * j:128 * (j + 1)],
                        x_sb[:, T * j + TQ * t: T * j + TQ * (t + 1)],
                        start=(j == 0), stop=(j == 1))
                nc.vector.tensor_copy(dst_sb[:, TQ * t:TQ * (t + 1)], ps[:])

            def proj_vT(i):
                """vT for s-tile i: out[t=128, o=128], scatter to vT_aug blocks."""
                ps = ps_sc.tile([128, 128], f32, tag="sc", name=f"vtps_{i}")
                for j in range(2):
                    nc.tensor.matmul(
                        ps[:],
                        x_sb[:, T * j + 128 * i: T * j + 128 * (i + 1)],
                        wvT_sb[:, 128 * j:128 * (j + 1)],
                        start=(j == 0), stop=(j == 1))
                for h in range(HG):
                    b = 4 * i + h
                    vc = VT_COL[h]
                    nc.vector.tensor_copy(
                        vT_aug[:, 128 * b + vc:128 * b + vc + DK],
                        ps[:, 32 * h:32 * (h + 1)])

            # pre-loop: just enough for the first exps
            proj_chunk(q_sb, wqT_sb, 0)
            proj_chunk(k_sb, wkT_sb, 0)

            # deferred projection work, drip-fed one task per attention iter
            tasks = [
                lambda: [proj_vT(i) for i in range(0, 4)],
                lambda: [proj_vT(i) for i in range(4, 8)],
                lambda: proj_chunk(k_sb, wkT_sb, 1),
                lambda: [proj_vT(i) for i in range(8, 12)],
                lambda: [proj_vT(i) for i in range(12, 16)],
                lambda: proj_chunk(k_sb, wkT_sb, 2),
                lambda: proj_chunk(q_sb, wqT_sb, 1),
                lambda: proj_chunk(k_sb, wkT_sb, 3),
                lambda: proj_chunk(q_sb, wqT_sb, 2),
                lambda: proj_chunk(q_sb, wqT_sb, 3),
            ]
            tasks.reverse()

            # ---------------- attention ----------------
            pv_ps = {}   # (c, p) -> [128, TQ] psum bank shared by heads 2p, 2p+1
            pending = []  # (c, i, [pT_A, pT_B]) awaiting PV emission

            def emit_pv(c, i, pTs):
                for h in range(HG):
                    pT = pTs[h // 2]
                    sl = pT[:, TQ * (h % 2):TQ * (h % 2 + 1)]
                    b = 4 * i + h
                    nc.tensor.matmul(
                        pv_ps[c, h // 2][:],
                        vT_aug[:, 128 * b:128 * (b + 1)],
                        sl,
                        start=(i == 0 and h % 2 == 0), stop=(i == NST - 1 and h % 2 == 1))

            def emit_epilogue(c):
                # Evacuate PV + sums rows (PSUM-sourced copies may shift
                # partitions) -> banks release after these; then recip+mul
                # fully in SBUF, all partition-aligned.
                pvc = epi_pool.tile([128, TQ], f32, tag="pvc", name=f"pvc_{c}")
                gat = epi_pool.tile([128, TQ], f32, tag="gat", name=f"gat_{c}")
                for h in range(HG):
                    bank = pv_ps[c, h // 2]
                    vc = VT_COL[h]
                    oc = ONES_COL[h]
                    nc.vector.tensor_copy(pvc[32 * h:32 * (h + 1), :],
                                          bank[vc:vc + DK, :])
                    nc.vector.tensor_copy(gat[32 * h:32 * (h + 1), :],
                                          bank[oc:oc + DK, :])
                recip = epi_pool.tile([128, TQ], f32, tag="recip", name=f"recip_{c}")
                rscr = epi_pool.tile([128, TQ], f32, tag="rscr", name=f"rscr_{c}")
                nc.vector.reciprocal_approx_accurate(recip[:], gat[:], rscr[:])
                outsb = epi_pool.tile([128, TQ], f32, tag="outsb", name=f"outsb_{c}")
                nc.vector.tensor_mul(outsb[:], pvc[:], recip[:])
                nc.sync.dma_start(out_d[:, TQ * c:TQ * (c + 1)], outsb[:])

            def flush_one():
                c0, i0, pTs = pending.pop(0)
                emit_pv(c0, i0, pTs)
                if i0 == NST - 1:
                    emit_epilogue(c0)

            for c in range(NCHUNK):
                for p in range(2):
                    pv_ps[c, p] = ps_pv.tile([128, TQ], f32, tag="pv",
                                             name=f"pv_{c}_{p}")
                for i in range(NST):
                    if tasks:
                        tasks.pop()()
                    pTs = []
                    for half in range(2):  # half 0: heads 0,1; half 1: heads 2,3
                        sc = ps_sc.tile([128, 2 * TQ], f32, tag="sc",
                                        name=f"sc_{c}_{i}_{half}")
                        for hh in range(2):
                            h = 2 * half + hh
                            nc.tensor.matmul(
                                sc[:, TQ * hh:TQ * (hh + 1)],
                                k_sb[32 * h:32 * (h + 1), 128 * i:128 * (i + 1)],
                                q_sb[32 * h:32 * (h + 1), TQ * c:TQ * (c + 1)],
                                start=True, stop=True,
                                tile_position=(32 * h, 0))
                        pT = pt_pool.tile([128, 2 * TQ], f32r, tag="pt",
                                          name=f"pt_{c}_{i}_{half}", bufs=8)
                        nc.scalar.activation(pT[:], sc[:], Exp,
                                             scale=float(INV_SQRT_DK))
                        pTs.append(pT)
                    pending.append((c, i, pTs))
                    if len(pending) > 3:
                        flush_one()
            while pending:
                flush_one()

    nc.compile()
    return nc


def _get_module():
    if "nc" not in _CACHE:
        _CACHE["nc"] = _build_module()
    return _CACHE["nc"]


def kernel(x, Wq, Wk, Wv):
    from concourse.bass_utils import run_bass_kernel_spmd

    nc = _get_module()
    x = np.ascontiguousarray(x, dtype=np.float32)
    in_maps = []
    for core in range(N_CORES):
        n, g = divmod(core, 2)
        rows = slice(128 * g, 128 * (g + 1))
        in_maps.append({
            "x": np.ascontiguousarray(x[n]),
            "wqT": np.ascontiguousarray(Wq[rows, :].T),
            "wkT": np.ascontiguousarray(Wk[rows, :].T),
            "wvT": np.ascontiguousarray(Wv[rows, :].T),
        })
    res = run_bass_kernel_spmd(nc, in_maps, core_ids=list(range(N_CORES)), trace=TRACE)
    LAST["res"] = res
    out = np.empty((4, 256, T), dtype=np.float32)
    for core in range(N_CORES):
        n, g = divmod(core, 2)
        out[n, 128 * g:128 * (g + 1), :] = res.results[core]["out"]
    return out


if __name__ == "__main__":
    _build_module()
    print("module built OK")


# revision 17
# speedup vs baseline: 1.3652x; 1.3270x over previous
"""MultiHeadAttn1D (N=4, C=256, T=2048, H=8, dk=dv=32) Trainium2 Bass kernel.

Sharding: 8 cores = 4 batches x 2 head-groups (4 heads each).
Per core: x_n [256,2048] -> q,k [128,2048] ([4h*32d, t]); augmented value
tiles vT_aug per (s-tile i, head h): [128s, 64] = [vT | ones] so one matmul
accumulates both PV and the softmax denominator (rows 32-63 = sums,
replicated); scores^T per (tq-chunk, s-tile) via K=32 row-tiled fp32r
matmuls -> PSUM; exp on ScalarE PSUM->SBUF (bottleneck engine, kept
saturated); normalize with DVE reciprocal+mul; DMA out.
"""

import sys

if "/opt/trn_rl_repo" not in sys.path:
    sys.path.insert(0, "/opt/trn_rl_repo")

import numpy as np

N_CORES = 8
C = 256          # in channels
T = 2048         # sequence length
HG = 4           # heads per core
DK = 32          # head dim
OC = 128         # output channels per core (HG * DK)
TQ = 512         # tq chunk width (1 psum bank)
NCHUNK = T // TQ          # 4
NST = T // 128            # 16 s-tiles
INV_SQRT_DK = 1.0 / np.sqrt(DK)
ONE_U32 = 0x3F800000      # 1.0f bit pattern
# vT_aug block layout per head (pair p=h//2 shares one PSUM bank; zero pad
# columns accumulate zeros into the partner head's rows, which is harmless):
#   h%2==0: vT at cols 0-31  (PV rows 0-31),  ones at cols 32-63  (sums rows 32-63)
#   h%2==1: vT at cols 64-95 (PV rows 64-95), ones at cols 96-127 (sums rows 96-127)
VT_COL = (0, 64, 0, 64)
ONES_COL = (32, 96, 32, 96)

USE_FP32R = True
TRACE = False
LAST = {}

_CACHE = {}


def _build_module():
    import contextlib

    from concourse import bacc, mybir
    import concourse.tile as tile

    f32 = mybir.dt.float32
    u32 = mybir.dt.uint32
    f32r = mybir.dt.float32r if USE_FP32R else mybir.dt.float32
    Exp = mybir.ActivationFunctionType.Exp

    nc = bacc.Bacc(
        "TRN2",
        target_bir_lowering=False,
        debug=False,
        num_devices=N_CORES,
    )

    x_d = nc.dram_tensor("x", [C, T], f32, kind="ExternalInput").ap()
    wqT_d = nc.dram_tensor("wqT", [C, OC], f32, kind="ExternalInput").ap()
    wkT_d = nc.dram_tensor("wkT", [C, OC], f32, kind="ExternalInput").ap()
    wvT_d = nc.dram_tensor("wvT", [C, OC], f32, kind="ExternalInput").ap()
    out_d = nc.dram_tensor("out", [OC, T], f32, kind="ExternalOutput").ap()

    with tile.TileContext(nc) as tc:
        ctx = contextlib.ExitStack()
        with ctx:
            persist = ctx.enter_context(tc.tile_pool(name="persist", bufs=1))
            pt_pool = ctx.enter_context(tc.tile_pool(name="pt", bufs=8))
            epi_pool = ctx.enter_context(tc.tile_pool(name="epi", bufs=2))
            # PSUM: scores 2x[128,1024] (4 banks, also hosts prologue
            # projection tiles via shared tag) + 2 shared PV banks x 2 chunks
            # in flight (so a new chunk never waits on the epilogue) = 8 banks.
            ps_sc = ctx.enter_context(tc.tile_pool(name="ps_sc", bufs=2, space="PSUM"))
            ps_pv = ctx.enter_context(tc.tile_pool(name="ps_pv", bufs=4, space="PSUM"))

            # ---------------- persistent SBUF ----------------
            wqT_sb = persist.tile([128, C], f32r)      # c-tile j at cols [128j:128j+128] = o
            wkT_sb = persist.tile([128, C], f32r)
            wvT_sb = persist.tile([128, C], f32r)
            x_sb = persist.tile([128, 2 * T], f32r)    # c-tile j at cols [T*j : T*j+T]
            q_sb = persist.tile([128, T], f32r)        # [o=4h*32, t]
            k_sb = persist.tile([128, T], f32r)
            # vT_aug: block b=(4i+h) at cols [128b:128b+128], zero-padded per
            # the VT_COL/ONES_COL layout above.
            vT_aug = persist.tile([128, 128 * HG * NST], f32r)
            x_stage = persist.tile([128, 2 * T], f32)
            w_stage = persist.tile([128, 3 * C], f32)

            # zero + ones init of vT_aug on GpSimd (DVE is busy staging x).
            vT_aug_u32 = vT_aug.bitcast(u32)
            nc.gpsimd.memset(vT_aug_u32[:], 0)
            v5 = vT_aug_u32.rearrange("p (i h w) -> p i h w", h=HG, w=128)
            for h in range(HG):
                oc = ONES_COL[h]
                nc.gpsimd.memset(v5[:, :, h, oc:oc + DK], ONE_U32)

            # ---------------- input DMAs + fp32r rounding casts ----------------
            def stage_w(idx, w_sb, w_d):
                for j in range(2):
                    sl = slice(C * idx + 128 * j, C * idx + 128 * (j + 1))
                    nc.sync.dma_start(w_stage[:, sl], w_d[128 * j:128 * (j + 1), :])
                    nc.vector.tensor_copy(w_sb[:, 128 * j:128 * (j + 1)], w_stage[:, sl])

            def stage_x(j, t):
                sl = slice(T * j + TQ * t, T * j + TQ * (t + 1))
                nc.sync.dma_start(x_stage[:, sl], x_d[128 * j:128 * (j + 1), TQ * t:TQ * (t + 1)])
                nc.vector.tensor_copy(x_sb[:, sl], x_stage[:, sl])

            # critical path first: q/k weights + x t-chunk 0
            stage_w(0, wqT_sb, wqT_d)
            stage_w(1, wkT_sb, wkT_d)
            stage_x(0, 0)
            stage_x(1, 0)
            stage_w(2, wvT_sb, wvT_d)
            for t in range(1, NCHUNK):
                stage_x(0, t)
                stage_x(1, t)

            # ---------------- projection helpers ----------------
            def proj_chunk(dst_sb, w_sb, t):
                """q/k projection for t-chunk: out[o, TQ] accumulated over 2 c-tiles."""
                ps = ps_sc.tile([128, TQ], f32, tag="sc", name=f"projps_{dst_sb.name}_{t}")
                for j in range(2):
                    nc.tensor.matmul(
                        ps[:],
                        w_sb[:, 128 * j:128 * (j + 1)],
                        x_sb[:, T * j + TQ * t: T * j + TQ * (t + 1)],
                        start=(j == 0), stop=(j == 1))
                nc.vector.tensor_copy(dst_sb[:, TQ * t:TQ * (t + 1)], ps[:])

            def proj_vT(i):
                """vT for s-tile i: out[t=128, o=128], scatter to vT_aug blocks."""
                ps = ps_sc.tile([128, 128], f32, tag="sc", name=f"vtps_{i}")
                for j in range(2):
                    nc.tensor.matmul(
                        ps[:],
                        x_sb[:, T * j + 128 * i: T * j + 128 * (i + 1)],
                        wvT_sb[:, 128 * j:128 * (j + 1)],
                        start=(j == 0), stop=(j == 1))
                for h in range(HG):
                    b = 4 * i + h
                    vc = VT_COL[h]
                    nc.vector.tensor_copy(
                        vT_aug[:, 128 * b + vc:128 * b + vc + DK],
                        ps[:, 32 * h:32 * (h + 1)])

            # pre-loop: just enough for the first exps
            proj_chunk(q_sb, wqT_sb, 0)
            proj_chunk(k_sb, wkT_sb, 0)

            # deferred projection work, drip-fed one task per attention iter
            tasks = [
                lambda: [proj_vT(i) for i in range(0, 4)],
                lambda: [proj_vT(i) for i in range(4, 8)],
                lambda: proj_chunk(k_sb, wkT_sb, 1),
                lambda: [proj_vT(i) for i in range(8, 12)],
                lambda: [proj_vT(i) for i in range(12, 16)],
                lambda: proj_chunk(k_sb, wkT_sb, 2),
                lambda: proj_chunk(q_sb, wqT_sb, 1),
                lambda: proj_chunk(k_sb, wkT_sb, 3),
                lambda: proj_chunk(q_sb, wqT_sb, 2),
                lambda: proj_chunk(q_sb, wqT_sb, 3),
            ]
            tasks.reverse()

            # ---------------- attention ----------------
            pv_ps = {}   # (c, p) -> [128, TQ] psum bank shared by heads 2p, 2p+1
            pending = []  # (c, i, [pT_A, pT_B]) awaiting PV emission

            def emit_pv(c, i, pTs):
                for h in range(HG):
                    pT = pTs[h // 2]
                    sl = pT[:, TQ * (h % 2):TQ * (h % 2 + 1)]
                    b = 4 * i + h
                    nc.tensor.matmul(
                        pv_ps[c, h // 2][:],
                        vT_aug[:, 128 * b:128 * (b + 1)],
                        sl,
                        start=(i == 0 and h % 2 == 0), stop=(i == NST - 1 and h % 2 == 1))

            def emit_epilogue(c):
                # Evacuate PV + sums rows (PSUM-sourced copies may shift
                # partitions) -> banks release after these; then recip+mul
                # fully in SBUF, all partition-aligned.
                pvc = epi_pool.tile([128, TQ], f32, tag="pvc", name=f"pvc_{c}")
                gat = epi_pool.tile([128, TQ], f32, tag="gat", name=f"gat_{c}")
                for h in range(HG):
                    bank = pv_ps[c, h // 2]
                    vc = VT_COL[h]
                    oc = ONES_COL[h]
                    nc.vector.tensor_copy(pvc[32 * h:32 * (h + 1), :],
                                          bank[vc:vc + DK, :])
                    nc.vector.tensor_copy(gat[32 * h:32 * (h + 1), :],
                                          bank[oc:oc + DK, :])
                recip = epi_pool.tile([128, TQ], f32, tag="recip", name=f"recip_{c}")
                rscr = epi_pool.tile([128, TQ], f32, tag="rscr", name=f"rscr_{c}")
                nc.vector.reciprocal_approx_accurate(recip[:], gat[:], rscr[:])
                outsb = epi_pool.tile([128, TQ], f32, tag="outsb", name=f"outsb_{c}")
                nc.vector.tensor_mul(outsb[:], pvc[:], recip[:])
                nc.sync.dma_start(out_d[:, TQ * c:TQ * (c + 1)], outsb[:])

            def flush_one():
                c0, i0, pTs = pending.pop(0)
                emit_pv(c0, i0, pTs)
                if i0 == NST - 1:
                    emit_epilogue(c0)

            for c in range(NCHUNK):
                for p in range(2):
                    pv_ps[c, p] = ps_pv.tile([128, TQ], f32, tag="pv",
                                             name=f"pv_{c}_{p}")
                for i in range(NST):
                    if tasks:
                        tasks.pop()()
                    pTs = []
                    for half in range(2):  # half 0: heads 0,1; half 1: heads 2,3
                        sc = ps_sc.tile([128, 2 * TQ], f32, tag="sc",
                                        name=f"sc_{c}_{i}_{half}")
                        for hh in range(2):
                            h = 2 * half + hh
                            nc.tensor.matmul(
                                sc[:, TQ * hh:TQ * (hh + 1)],
                                k_sb[32 * h:32 * (h + 1), 128 * i:128 * (i + 1)],
                                q_sb[32 * h:32 * (h + 1), TQ * c:TQ * (c + 1)],
                                start=True, stop=True,
                                tile_position=(32 * h, 0))
                        pT = pt_pool.tile([128, 2 * TQ], f32r, tag="pt",
                                          name=f"pt_{c}_{i}_{half}", bufs=8)
                        nc.scalar.activation(pT[:], sc[:], Exp,
                                             scale=float(INV_SQRT_DK))
                        pTs.append(pT)
                    pending.append((c, i, pTs))
                    if len(pending) > 3:
                        flush_one()
            while pending:
                flush_one()

    nc.compile()
    return nc


def _get_module():
    if "nc" not in _CACHE:
        _CACHE["nc"] = _build_module()
    return _CACHE["nc"]


def kernel(x, Wq, Wk, Wv):
    from concourse.bass_utils import run_bass_kernel_spmd

    nc = _get_module()
    x = np.ascontiguousarray(x, dtype=np.float32)
    in_maps = []
    for core in range(N_CORES):
        n, g = divmod(core, 2)
        rows = slice(128 * g, 128 * (g + 1))
        in_maps.append({
            "x": np.ascontiguousarray(x[n]),
            "wqT": np.ascontiguousarray(Wq[rows, :].T),
            "wkT": np.ascontiguousarray(Wk[rows, :].T),
            "wvT": np.ascontiguousarray(Wv[rows, :].T),
        })
    res = run_bass_kernel_spmd(nc, in_maps, core_ids=list(range(N_CORES)), trace=TRACE)
    LAST["res"] = res
    out = np.empty((4, 256, T), dtype=np.float32)
    for core in range(N_CORES):
        n, g = divmod(core, 2)
        out[n, 128 * g:128 * (g + 1), :] = res.results[core]["out"]
    return out


if __name__ == "__main__":
    _build_module()
    print("module built OK")
